# revision 2
# baseline (speedup 1.0000x reference)
"""Trainium2 Bass kernel for BinaryNN forward (binary conv net + log_softmax).

Contract: kernel(**inputs) takes FULL unsharded inputs
  x     [8192, 1, 28, 28] f32
  w1    [16, 1, 3, 3]     f32
  w2    [16, 16, 3, 3]    f32
  fc_w  [10, 2304]        f32
returns [8192, 10] f32 log_softmax logits.

Strategy: pure data parallel over 8 NeuronCores (batch 1024/core). All
binarization of weights happens on host (tiny); convolutions are lowered to
dense TensorEngine matmuls via Toeplitz "x-window" weight matrices with the
batch dimension streaming as matmul columns. All intermediate activations are
exact small integers, so fp8/bf16 storage is bit-exact and sign() can run as
either ACT Sign or a DVE integer clamp.

Per-core pipeline (B=1024 split into 2 halves of 512):
  conv1: K=30 [(dy,xi) window of 10] x M=128 [(c=16) x (xr=8)] -> PSUM chunks
         [128, 512], sign -> A1 fp8 [128, 26y * 512b]   (4 overlapping
         x-windows of width 8 covering out width 26)
  conv2: per window, per out-row y: 3 dy-accumulated MMs K=128 x M=112
         (out channels x {even xr | odd xr} split at partition 64) ->
         sign -> bf16; 2x2 avg-pool+sign == sign of 4-term integer sum:
         one free-dim add (y pairs) + one partition-offset add (x pairs),
         clamp/sign -> PSw fp8 [48, 512]
  fc:    48 accumulating MMs K=48 x M=10, N=512 -> logits PSUM [10, 512]
  softmax: PE-transpose to [128, 10], Exp with accum_out, Ln, fused
         (x - max - logsum) via one tensor_scalar.
"""

import functools
import numpy as np
import ml_dtypes

N_CORES = 8
B_TOTAL = 8192
B = B_TOTAL // N_CORES  # 1024 per core
BH = 512                # half-batch processed per outer iteration
THRESH = 0.2

FP8 = ml_dtypes.float8_e4m3


# ----------------------------------------------------------------------------
# Device program (built once, cached)
# ----------------------------------------------------------------------------

@functools.lru_cache(maxsize=1)
def _build_program():
    from contextlib import ExitStack
    import concourse.bass as bass
    import concourse.tile as tile
    import concourse.mybir as mybir
    from concourse import bacc

    f32 = mybir.dt.float32
    bf16 = mybir.dt.bfloat16
    fp8 = mybir.dt.float8e4
    AF = mybir.ActivationFunctionType
    ALU = mybir.AluOpType
    AX = mybir.AxisListType

    nc = bacc.Bacc(
        "TRN2",
        target_bir_lowering=False,
        debug=False,
        num_devices=N_CORES,
    )

    xq_t = nc.dram_tensor("xq", [28, 28, B], fp8, kind="ExternalInput")
    wl1_t = nc.dram_tensor("wl1", [30, 128], fp8, kind="ExternalInput")
    wl2_t = nc.dram_tensor("wl2", [128, 384], fp8, kind="ExternalInput")
    wfc_t = nc.dram_tensor("wfc", [48, 480], fp8, kind="ExternalInput")
    idt_t = nc.dram_tensor("ident", [10, 10], f32, kind="ExternalInput")
    out_t = nc.dram_tensor("out", [B, 10], f32, kind="ExternalOutput")

    Y1 = 26          # conv1 out rows
    NW = 4           # x-windows
    COLS1 = Y1 * BH  # A1 free size

    def emit(ctx, tc):
        wpool = ctx.enter_context(tc.tile_pool(name="weights", bufs=1))
        rhs1_pool = ctx.enter_context(tc.tile_pool(name="rhs1", bufs=2))
        a1_pool = ctx.enter_context(tc.tile_pool(name="a1", bufs=2))
        s2_pool = ctx.enter_context(tc.tile_pool(name="s2", bufs=3))
        t_pool = ctx.enter_context(tc.tile_pool(name="tp", bufs=3))
        p2_pool = ctx.enter_context(tc.tile_pool(name="p2", bufs=3))
        psw_pool = ctx.enter_context(tc.tile_pool(name="psw", bufs=3))
        sm_pool = ctx.enter_context(tc.tile_pool(name="sm", bufs=10))
        ps1_pool = ctx.enter_context(tc.tile_pool(name="ps1", bufs=2, space="PSUM"))
        ps2_pool = ctx.enter_context(tc.tile_pool(name="ps2", bufs=2, space="PSUM"))
        lg_pool = ctx.enter_context(tc.tile_pool(name="lg", bufs=1, space="PSUM"))
        pt_pool = ctx.enter_context(tc.tile_pool(name="pt", bufs=1, space="PSUM"))

        wl1 = wpool.tile([30, 128], fp8)
        nc.gpsimd.dma_start(wl1[:], wl1_t.ap())
        wl2 = wpool.tile([128, 384], fp8)
        nc.gpsimd.dma_start(wl2[:], wl2_t.ap())
        wfc = wpool.tile([48, 480], fp8)
        nc.gpsimd.dma_start(wfc[:], wfc_t.ap())
        idt = wpool.tile([10, 10], f32)
        nc.gpsimd.dma_start(idt[:], idt_t.ap())
        lsb = wpool.tile([10, B], f32)  # logits staging, both halves

        def sign_to(dst, src, use_act):
            # src holds exact integers -> clamp(-1, 1) == sign()
            if use_act:
                nc.scalar.sign(dst, src)
            else:
                nc.vector.tensor_scalar(dst, src, -1.0, 1.0, ALU.max, ALU.min)

        def emit_conv1(h, w):
            rhs1 = rhs1_pool.tile([30, COLS1], fp8, tag="rhs1")
            for dyi in range(3):
                src = bass.AP(
                    xq_t,
                    (6 * w + dyi * 28) * B + h * BH,
                    [[B, 10], [28 * B, Y1], [1, BH]],
                )
                nc.sync.dma_start(rhs1[dyi * 10:(dyi + 1) * 10, :], src)
            a1 = a1_pool.tile([128, COLS1], fp8, tag="a1")
            for yc in range(Y1):
                ps1 = ps1_pool.tile([128, BH], f32, tag="ps1")
                nc.tensor.matmul(
                    ps1[:], wl1[:], rhs1[:, yc * BH:(yc + 1) * BH],
                    start=True, stop=True,
                )
                sign_to(a1[:, yc * BH:(yc + 1) * BH], ps1[:], yc % 2 == 0)
            return a1

        def emit_fc(lg, k, psw):
            nc.tensor.matmul(
                lg[:], wfc[:, k * 10:(k + 1) * 10], psw[:],
                start=(k == 0), stop=(k == 47),
            )

        def emit_rest(h, w, a1, lg, fc_pending):
            for py in range(12):
                ps2 = ps2_pool.tile([128, 2 * BH], f32, tag="ps2")
                for hy in range(2):
                    y = 2 * py + hy
                    # dy 0+1 fused in one fp8 DoubleRow matmul (K=256 virtual)
                    nc.tensor.matmul(
                        ps2[:, hy * BH:(hy + 1) * BH],
                        wl2[:, 0:256].rearrange("p (two m) -> p two m", two=2),
                        a1[:, y * BH:(y + 2) * BH].rearrange(
                            "p (two n) -> p two n", two=2),
                        start=True, stop=False,
                        perf_mode=mybir.MatmulPerfMode.DoubleRow,
                    )
                    nc.tensor.matmul(
                        ps2[:, hy * BH:(hy + 1) * BH],
                        wl2[:, 256:384],
                        a1[:, (y + 2) * BH:(y + 3) * BH],
                        start=False, stop=True,
                    )
                # fc matmuls run 2 py-slots behind so PE never waits on the
                # sign/pool chain
                while len(fc_pending) > 2:
                    emit_fc(lg, *fc_pending.pop(0))
                s2 = s2_pool.tile([128, 2 * BH], bf16, tag="s2")
                sign_to(s2[:], ps2[:], py % 4 != 0)
                # pool-y for even/odd x separately (inputs of each add share a
                # base partition; the odd add writes partition-shifted 64->0)
                te = t_pool.tile([48, BH], bf16, tag="te")
                nc.vector.tensor_add(te[:], s2[0:48, 0:BH], s2[0:48, BH:2 * BH])
                to = t_pool.tile([48, BH], bf16, tag="to")
                nc.vector.tensor_add(to[:], s2[64:112, 0:BH],
                                     s2[64:112, BH:2 * BH])
                p2 = p2_pool.tile([48, BH], bf16, tag="p2")
                nc.vector.tensor_add(p2[:], te[:], to[:])
                psw = psw_pool.tile([48, BH], fp8, tag="psw")
                sign_to(psw[:], p2[:], True)
                fc_pending.append((w * 12 + py, psw))

        out_ap = out_t.ap()

        def emit_softmax(h):
            # log_softmax for this half's 4 chunks of 128 rows; grouped by
            # activation function so ACT reloads its table at most twice
            lqs, nms, ses, lss = [], [], [], []
            for qq in range(4):
                q = 4 * h + qq
                pt = pt_pool.tile([128, 10], f32, tag="pt")
                nc.tensor.transpose(pt[:], lsb[:, q * 128:(q + 1) * 128],
                                    idt[:])
                lq = sm_pool.tile([128, 10], f32, tag=f"lq{q}")
                nc.vector.tensor_copy(lq[:], pt[:])
                nm = sm_pool.tile([128, 1], f32, tag=f"nm{q}")
                nc.vector.reduce_max(nm[:], lq[:], axis=AX.X, negate=True)
                lqs.append(lq)
                nms.append(nm)
            for qq in range(4):
                q = 4 * h + qq
                scr = sm_pool.tile([128, 10], f32, tag="scr", bufs=2)
                se = sm_pool.tile([128, 1], f32, tag=f"se{q}")
                nc.scalar.activation(scr[:], lqs[qq][:], AF.Exp,
                                     bias=nms[qq][:], accum_out=se[:])
                ses.append(se)
            for qq in range(4):
                q = 4 * h + qq
                ls = sm_pool.tile([128, 1], f32, tag=f"ls{q}")
                nc.scalar.activation(ls[:], ses[qq][:], AF.Ln)
                lss.append(ls)
            for qq in range(4):
                q = 4 * h + qq
                o = sm_pool.tile([128, 10], f32, tag="o", bufs=2)
                nc.vector.tensor_scalar(o[:], lqs[qq][:], nms[qq][:],
                                        lss[qq][:], ALU.add, ALU.subtract)
                nc.sync.dma_start(out_ap[q * 128:(q + 1) * 128, :], o[:])

        for h in range(2):
            lg = lg_pool.tile([10, BH], f32, tag="lg")
            a1_prev = None
            fc_pending = []
            # software-pipeline: conv1 of window w+1 is emitted before
            # conv2/pool/fc of window w so PE never waits on sign() latency
            for w in range(NW):
                a1 = emit_conv1(h, w)
                if a1_prev is not None:
                    emit_rest(h, w - 1, a1_prev, lg, fc_pending)
                a1_prev = a1
            emit_rest(h, NW - 1, a1_prev, lg, fc_pending)
            while fc_pending:
                emit_fc(lg, *fc_pending.pop(0))
            nc.vector.tensor_copy(lsb[:, h * BH:(h + 1) * BH], lg[:])
            emit_softmax(h)

    with tile.TileContext(nc) as tc:
        with ExitStack() as ctx:
            emit(ctx, tc)

    nc.compile()
    return nc


# ----------------------------------------------------------------------------
# Host-side weight packing
# ----------------------------------------------------------------------------

def _pack_weights(w1, w2, fc_w):
    w1s = np.sign(w1[:, 0].astype(np.float32))   # [16,3,3]
    w2s = np.sign(w2.astype(np.float32))         # [16,16,3,3]
    fcs = np.sign(fc_w.astype(np.float32))       # [10,2304]

    # conv1 Toeplitz: rows k=(dy,xi in 0..9), cols m=(o,xr in 0..7)
    L1 = np.zeros((30, 128), np.float32)
    for o in range(16):
        for xr in range(8):
            for dy in range(3):
                for dx in range(3):
                    L1[dy * 10 + xr + dx, o * 8 + xr] = w1s[o, dy, dx]

    # conv2 Toeplitz per dy: rows k=(c,xi in 0..7), cols j:
    #   j in [0,48):   o=j//3, xr=2*(j%3)      (even out-x)
    #   j in [64,112): o=(j-64)//3, xr=2*((j-64)%3)+1  (odd out-x)
    L2 = np.zeros((128, 384), np.float32)
    for dy in range(3):
        for c in range(16):
            for xi in range(8):
                k = c * 8 + xi
                for j in range(112):
                    if j < 48:
                        o, xr = j // 3, 2 * (j % 3)
                    elif j >= 64:
                        o, xr = (j - 64) // 3, 2 * ((j - 64) % 3) + 1
                    else:
                        continue
                    dx = xi - xr
                    if 0 <= dx <= 2:
                        if dy < 2:
                            L2[k, dy * 128 + j] = w2s[o, c, dy, dx]
                        else:
                            L2[k, 256 + j] = w2s[o, c, dy, dx]

    # fc chunks: per (w,py): rows p=(o,pxl), cols=class
    Lfc = np.zeros((48, 480), np.float32)
    for w in range(4):
        for py in range(12):
            k = w * 12 + py
            for p in range(48):
                o, pxl = p // 3, p % 3
                feat = o * 144 + py * 12 + 3 * w + pxl
                Lfc[p, k * 10:(k + 1) * 10] = fcs[:, feat]

    return L1.astype(FP8), L2.astype(FP8), Lfc.astype(FP8)


def _prep_inputs(x, w1, w2, fc_w):
    xq = np.where(x.reshape(B_TOTAL, 28, 28) >= THRESH, 1.0, -1.0)
    xq_t = np.ascontiguousarray(np.transpose(xq, (1, 2, 0))).astype(FP8)
    L1, L2, Lfc = _pack_weights(w1, w2, fc_w)
    ident = np.eye(10, dtype=np.float32)
    in_maps = []
    for i in range(N_CORES):
        in_maps.append({
            "xq": np.ascontiguousarray(xq_t[:, :, i * B:(i + 1) * B]),
            "wl1": L1, "wl2": L2, "wfc": Lfc, "ident": ident,
        })
    return in_maps


# ----------------------------------------------------------------------------
# Entry point
# ----------------------------------------------------------------------------

TRACE = False
LAST_RESULTS = None


def kernel(x, w1, w2, fc_w):
    global LAST_RESULTS
    from concourse.bass_utils import run_bass_kernel_spmd

    x = np.asarray(x)
    in_maps = _prep_inputs(x, np.asarray(w1), np.asarray(w2), np.asarray(fc_w))
    nc = _build_program()
    res = run_bass_kernel_spmd(nc, in_maps, list(range(N_CORES)), trace=TRACE)
    LAST_RESULTS = res
    out = np.concatenate(
        [np.asarray(res.results[i]["out"]) for i in range(N_CORES)], axis=0
    )
    return out.astype(np.float32)



# revision 4
# speedup vs baseline: 1.6088x; 1.6088x over previous
"""Trainium2 Bass kernel for BinaryNN forward (binary conv net + log_softmax).

Contract: kernel(**inputs) takes FULL unsharded inputs
  x     [8192, 1, 28, 28] f32
  w1    [16, 1, 3, 3]     f32
  w2    [16, 16, 3, 3]    f32
  fc_w  [10, 2304]        f32
returns [8192, 10] f32 log_softmax logits.

Strategy: pure data parallel over 8 NeuronCores (batch 1024/core), conv lowered
to fp8 TensorEngine matmuls. v2 design from HW microbenchmarks:
  - conv1 (K=30) row-tiled 2x: two concurrent matmuls in PE row-groups 0/32
    (window data replicated at partition bases 0 and 32, pre-laid-out on host
    so the device DMA is a contiguous burst).
  - conv2: fp8 DoubleRow (dy0,dy1 as 2 K-planes in one pass, 2 planes/cycle)
    + single pass for dy2, per output row, N=512.
  - 2x2 avg-pool+sign: pool sums computed on PE as a DoubleRow matmul with a
    0/1 matrix (y-pair in the 2 planes, x-pair folded into the matrix),
    replacing the DVE add chain.
  - fc: chunk-pair DoubleRow matmuls (K=96 virtual) in an end-of-half burst so
    the logits PSUM bank borrows the conv2 pool's rotation.
  - every sign() is one PSUM->SBUF clamp/Sign instruction on [*,1024] tiles,
    alternating ACT and DVE to split the elementwise wall across both engines.
PSUM: conv1 pool 2x[128,1024] (4 banks) + conv2/pool/fc/transpose shared pool
2x[128,1024] (4 banks).
"""

import functools
import numpy as np
import ml_dtypes

N_CORES = 8
B_TOTAL = 8192
B = B_TOTAL // N_CORES  # 1024 per core
BH = 512                # half-batch processed per outer iteration
THRESH = 0.2

FP8 = ml_dtypes.float8_e4m3


# ----------------------------------------------------------------------------
# Device program (built once, cached)
# ----------------------------------------------------------------------------

@functools.lru_cache(maxsize=1)
def _build_program():
    from contextlib import ExitStack
    import concourse.bass as bass
    import concourse.tile as tile
    import concourse.mybir as mybir
    from concourse import bacc

    f32 = mybir.dt.float32
    fp8 = mybir.dt.float8e4
    AF = mybir.ActivationFunctionType
    ALU = mybir.AluOpType
    AX = mybir.AxisListType
    DR = mybir.MatmulPerfMode.DoubleRow

    nc = bacc.Bacc(
        "TRN2",
        target_bir_lowering=False,
        debug=False,
        num_devices=N_CORES,
    )

    Y1 = 26          # conv1 out rows
    NW = 4           # x-windows
    WCOLS = Y1 * BH  # per-(w,h) window free size (13312)

    # host-prepacked, 2-replica conv1 window blocks: [8, 64, 26*512]
    xqr_t = nc.dram_tensor("xqr", [8, 64, WCOLS], fp8, kind="ExternalInput")
    wl1_t = nc.dram_tensor("wl1", [64, 128], fp8, kind="ExternalInput")
    wl2_t = nc.dram_tensor("wl2", [128, 384], fp8, kind="ExternalInput")
    wpl_t = nc.dram_tensor("wpl", [128, 96], fp8, kind="ExternalInput")
    wfc_t = nc.dram_tensor("wfc", [48, 768], fp8, kind="ExternalInput")
    idt_t = nc.dram_tensor("ident", [10, 10], f32, kind="ExternalInput")
    out_t = nc.dram_tensor("out", [B, 10], f32, kind="ExternalOutput")

    def emit(ctx, tc):
        wpool = ctx.enter_context(tc.tile_pool(name="weights", bufs=1))
        rhs1_pool = ctx.enter_context(tc.tile_pool(name="rhs1", bufs=2))
        a1_pool = ctx.enter_context(tc.tile_pool(name="a1", bufs=2))
        s2_pool = ctx.enter_context(tc.tile_pool(name="s2", bufs=2))
        psw_pool = ctx.enter_context(tc.tile_pool(name="psw", bufs=2))
        sm_pool = ctx.enter_context(tc.tile_pool(name="sm", bufs=10))
        ps1_pool = ctx.enter_context(tc.tile_pool(name="ps1", bufs=2, space="PSUM"))
        ps2_pool = ctx.enter_context(tc.tile_pool(name="ps2", bufs=2, space="PSUM"))

        wl1 = wpool.tile([64, 128], fp8)
        nc.gpsimd.dma_start(wl1[:], wl1_t.ap())
        wl2 = wpool.tile([128, 384], fp8)
        nc.gpsimd.dma_start(wl2[:], wl2_t.ap())
        wpl = wpool.tile([128, 96], fp8)
        nc.gpsimd.dma_start(wpl[:], wpl_t.ap())
        wfc = wpool.tile([48, 768], fp8)
        nc.gpsimd.dma_start(wfc[:], wfc_t.ap())
        idt = wpool.tile([10, 10], f32)
        nc.gpsimd.dma_start(idt[:], idt_t.ap())
        lsb = wpool.tile([10, B], f32)  # logits staging, both halves

        eng = [0]

        def sign_to(dst, src):
            # src holds exact integers -> clamp(-1,1) == sign(); alternate
            # engines to split the PSUM->SBUF wall
            eng[0] ^= 1
            if eng[0]:
                nc.scalar.sign(dst, src)
            else:
                nc.vector.tensor_scalar(dst, src, -1.0, 1.0, ALU.max, ALU.min)

        def dma_rhs1(h, w):
            blk = h * NW + w
            rhs1 = rhs1_pool.tile([64, WCOLS], fp8, tag="rhs1", name="rhs1")
            for g in range(7):
                c0 = g * 2048
                cn = min(2048, WCOLS - c0)
                src = bass.AP(
                    xqr_t,
                    blk * 64 * WCOLS + c0,
                    [[WCOLS, 64], [1, cn]],
                )
                nc.sync.dma_start(rhs1[0:64, c0:c0 + cn], src)
            return rhs1

        def emit_conv1(rhs1, a1):
            # 13 row-tiled packs of 2 (y, y+1)
            for p in range(13):
                ps1 = ps1_pool.tile([128, 1024], f32, tag="ps1", name="ps1")
                for i in range(2):
                    y = 2 * p + i
                    nc.tensor.matmul(
                        ps1[:, i * 512:(i + 1) * 512],
                        wl1[32 * i:32 * i + 30, :],
                        rhs1[32 * i:32 * i + 30, y * 512:(y + 1) * 512],
                        start=True, stop=True, tile_position=(32 * i, 0),
                    )
                sign_to(a1[:, p * 1024:(p + 1) * 1024], ps1[:])

        def emit_conv2_pool(w, a1, s2, pswh):
            for q in range(6):        # py pairs
                pool_srcs = []
                for py in (2 * q, 2 * q + 1):
                    ps2 = ps2_pool.tile([128, 1024], f32, tag="c2", name="c2")
                    for hy in range(2):
                        y = 2 * py + hy
                        nc.tensor.matmul(
                            ps2[:, hy * 512:(hy + 1) * 512],
                            wl2[:, 0:256].rearrange("p (two m) -> p two m", two=2),
                            a1[:, y * 512:(y + 2) * 512].rearrange(
                                "p (two n) -> p two n", two=2),
                            start=True, stop=False, perf_mode=DR,
                        )
                        nc.tensor.matmul(
                            ps2[:, hy * 512:(hy + 1) * 512],
                            wl2[:, 256:384],
                            a1[:, (y + 2) * 512:(y + 3) * 512],
                            start=False, stop=True,
                        )
                    sc = s2[:, py * 1024:(py + 1) * 1024]
                    sign_to(sc, ps2[:])
                    pool_srcs.append(sc)
                # pool: DR matmul per py (y-pair = 2 planes, x-pair in matrix)
                psp = ps2_pool.tile([128, 1024], f32, tag="c2", name="psp")
                for j, sc in enumerate(pool_srcs):
                    nc.tensor.matmul(
                        psp[0:48, j * 512:(j + 1) * 512],
                        wpl[:].rearrange("p (two m) -> p two m", two=2),
                        sc.rearrange("p (two n) -> p two n", two=2),
                        start=True, stop=True, perf_mode=DR,
                    )
                sign_to(pswh[0:48, (w * 6 + q) * 1024:(w * 6 + q + 1) * 1024],
                        psp[0:48, :])

        def emit_fc_softmax(h, pswh):
            lgt = ps2_pool.tile([128, 1024], f32, tag="c2", name="lgt")
            lg = lgt[0:16, 0:512]
            for j in range(24):
                nc.tensor.matmul(
                    lg,
                    wfc[:, j * 32:(j + 1) * 32].rearrange(
                        "p (two m) -> p two m", two=2),
                    pswh[0:48, j * 1024:(j + 1) * 1024].rearrange(
                        "p (two n) -> p two n", two=2),
                    start=(j == 0), stop=(j == 23), perf_mode=DR,
                )
            nc.vector.tensor_copy(lsb[:, h * BH:(h + 1) * BH], lg[0:10, :])
            # log_softmax on 4 chunks of 128 images, ACT funcs grouped
            lqs, nms, ses = [], [], []
            for qq in range(4):
                q = 4 * h + qq
                ptt = ps2_pool.tile([128, 1024], f32, tag="c2", name="ptt")
                nc.tensor.transpose(ptt[0:128, 0:10],
                                    lsb[:, q * 128:(q + 1) * 128], idt[:])
                lq = sm_pool.tile([128, 10], f32, tag=f"lq{qq}", name="lq")
                nc.vector.tensor_copy(lq[:], ptt[0:128, 0:10])
                nm = sm_pool.tile([128, 1], f32, tag=f"nm{qq}", name="nm")
                nc.vector.reduce_max(nm[:], lq[:], axis=AX.X, negate=True)
                lqs.append(lq)
                nms.append(nm)
            for qq in range(4):
                scr = sm_pool.tile([128, 10], f32, tag="scr", name="scr", bufs=2)
                se = sm_pool.tile([128, 1], f32, tag=f"se{qq}", name="se")
                nc.scalar.activation(scr[:], lqs[qq][:], AF.Exp,
                                     bias=nms[qq][:], accum_out=se[:])
                ses.append(se)
            lss = []
            for qq in range(4):
                ls = sm_pool.tile([128, 1], f32, tag=f"ls{qq}", name="ls")
                nc.scalar.activation(ls[:], ses[qq][:], AF.Ln)
                lss.append(ls)
            out_ap = out_t.ap()
            for qq in range(4):
                q = 4 * h + qq
                o = sm_pool.tile([128, 10], f32, tag="o", name="o", bufs=2)
                nc.vector.tensor_scalar(o[:], lqs[qq][:], nms[qq][:],
                                        lss[qq][:], ALU.add, ALU.subtract)
                nc.sync.dma_start(out_ap[q * 128:(q + 1) * 128, :], o[:])

        for h in range(2):
            pswh = psw_pool.tile([48, 24 * 1024], fp8, tag="pswh", name="pswh")
            rhs1 = dma_rhs1(h, 0)
            prev = None  # (w, a1, s2)
            for w in range(NW):
                a1 = a1_pool.tile([128, WCOLS], fp8, tag="a1", name="a1")
                emit_conv1(rhs1, a1)
                if w + 1 < NW:
                    rhs1 = dma_rhs1(h, w + 1)
                if prev is not None:
                    emit_conv2_pool(prev[0], prev[1], prev[2], pswh)
                s2 = s2_pool.tile([128, 12 * 1024], fp8, tag="s2", name="s2")
                prev = (w, a1, s2)
            emit_conv2_pool(prev[0], prev[1], prev[2], pswh)
            emit_fc_softmax(h, pswh)

    with tile.TileContext(nc) as tc:
        with ExitStack() as ctx:
            emit(ctx, tc)

    nc.compile()
    return nc


# ----------------------------------------------------------------------------
# Host-side packing
# ----------------------------------------------------------------------------

def _pack_weights(w1, w2, fc_w):
    w1s = np.sign(w1[:, 0].astype(np.float32))   # [16,3,3]
    w2s = np.sign(w2.astype(np.float32))         # [16,16,3,3]
    fcs = np.sign(fc_w.astype(np.float32))       # [10,2304]

    # conv1 Toeplitz: rows k=(dy,xi in 0..9), cols m=(o,xr in 0..7);
    # two replicas at partition bases 0 and 32 for row-tiling
    L1 = np.zeros((64, 128), np.float32)
    for o in range(16):
        for xr in range(8):
            for dy in range(3):
                for dx in range(3):
                    v = w1s[o, dy, dx]
                    L1[dy * 10 + xr + dx, o * 8 + xr] = v
                    L1[32 + dy * 10 + xr + dx, o * 8 + xr] = v

    # conv2 Toeplitz per dy: rows k=(c,xi in 0..7), cols j:
    #   j in [0,48):   o=j//3, xr=2*(j%3)      (even out-x)
    #   j in [64,112): o=(j-64)//3, xr=2*((j-64)%3)+1  (odd out-x)
    L2 = np.zeros((128, 384), np.float32)
    for dy in range(3):
        for c in range(16):
            for xi in range(8):
                k = c * 8 + xi
                for j in range(112):
                    if j < 48:
                        o, xr = j // 3, 2 * (j % 3)
                    elif j >= 64:
                        o, xr = (j - 64) // 3, 2 * ((j - 64) % 3) + 1
                    else:
                        continue
                    dx = xi - xr
                    if 0 <= dx <= 2:
                        if dy < 2:
                            L2[k, dy * 128 + j] = w2s[o, c, dy, dx]
                        else:
                            L2[k, 256 + j] = w2s[o, c, dy, dx]

    # pool matrix: out m=(o,pxl in 0..2) sums s2 partitions (even j, odd j);
    # DR: plane 0 and plane 1 identical (y-pair via rhs planes)
    P = np.zeros((128, 96), np.float32)
    for o in range(16):
        for pxl in range(3):
            m = o * 3 + pxl
            je = o * 3 + pxl          # even-x partition (j in [0,48))
            jo = 64 + o * 3 + pxl     # odd-x partition  (j in [64,112))
            for pl in range(2):
                P[je, pl * 48 + m] = 1.0
                P[jo, pl * 48 + m] = 1.0

    # fc chunk-pairs: pair j=(w*6+q) = chunks k0=(w,2q), k1=(w,2q+1),
    # k=(w,py): feature(p=(o,pxl)) = o*144 + py*12 + 3*w + pxl
    Lfc = np.zeros((48, 768), np.float32)
    for w in range(4):
        for q in range(6):
            j = w * 6 + q
            for pl in range(2):
                py = 2 * q + pl
                for p in range(48):
                    o, pxl = p // 3, p % 3
                    feat = o * 144 + py * 12 + 3 * w + pxl
                    Lfc[p, j * 32 + pl * 16:j * 32 + pl * 16 + 10] = fcs[:, feat]

    return (L1.astype(FP8), L2.astype(FP8), P.astype(FP8), Lfc.astype(FP8))


def _prep_inputs(x, w1, w2, fc_w):
    Y1 = 26
    xq = np.where(x.reshape(B_TOTAL, 28, 28) >= THRESH, 1.0, -1.0)
    xq_t = np.transpose(xq, (1, 2, 0)).astype(FP8)  # [28, 28, B_TOTAL]
    L1, L2, P, Lfc = _pack_weights(w1, w2, fc_w)
    ident = np.eye(10, dtype=np.float32)

    in_maps = []
    for i in range(N_CORES):
        xc = xq_t[:, :, i * B:(i + 1) * B]  # [28, 28, 1024]
        # window blocks: blk=(h,w): [64, 26*512] with taps (dy,xi) replicated
        # at partition bases 0 and 32; col (y,b) holds xq[y+dy, 6w+xi, h*512+b]
        xqr = np.zeros((8, 64, Y1 * BH), FP8)
        for h in range(2):
            for w in range(4):
                blk = h * 4 + w
                # [3dy, 10xi, 26y, 512b]
                base = np.stack([
                    np.stack([
                        xc[dy:dy + Y1, 6 * w + xi, h * BH:(h + 1) * BH]
                        for xi in range(10)
                    ], axis=0)
                    for dy in range(3)
                ], axis=0)
                flat = base.reshape(30, Y1 * BH)
                xqr[blk, 0:30] = flat
                xqr[blk, 32:62] = flat
        in_maps.append({
            "xqr": xqr, "wl1": L1, "wl2": L2, "wpl": P, "wfc": Lfc,
            "ident": ident,
        })
    return in_maps


# ----------------------------------------------------------------------------
# Entry point
# ----------------------------------------------------------------------------

TRACE = False
LAST_RESULTS = None


def kernel(x, w1, w2, fc_w):
    global LAST_RESULTS
    from concourse.bass_utils import run_bass_kernel_spmd

    x = np.asarray(x)
    in_maps = _prep_inputs(x, np.asarray(w1), np.asarray(w2), np.asarray(fc_w))
    nc = _build_program()
    res = run_bass_kernel_spmd(nc, in_maps, list(range(N_CORES)), trace=TRACE)
    LAST_RESULTS = res
    out = np.concatenate(
        [np.asarray(res.results[i]["out"]) for i in range(N_CORES)], axis=0
    )
    return out.astype(np.float32)


# revision 5
# speedup vs baseline: 1.6247x; 1.0099x over previous
"""Trainium2 Bass kernel for BinaryNN forward (binary conv net + log_softmax).

Contract: kernel(**inputs) takes FULL unsharded inputs
  x     [8192, 1, 28, 28] f32
  w1    [16, 1, 3, 3]     f32
  w2    [16, 16, 3, 3]    f32
  fc_w  [10, 2304]        f32
returns [8192, 10] f32 log_softmax logits.

Strategy: pure data parallel over 8 NeuronCores (batch 1024/core), conv lowered
to fp8 TensorEngine matmuls. v2 design from HW microbenchmarks:
  - conv1 (K=30) row-tiled 2x: two concurrent matmuls in PE row-groups 0/32
    (window data replicated at partition bases 0 and 32, pre-laid-out on host
    so the device DMA is a contiguous burst).
  - conv2: fp8 DoubleRow (dy0,dy1 as 2 K-planes in one pass, 2 planes/cycle)
    + single pass for dy2, per output row, N=512.
  - 2x2 avg-pool+sign: pool sums computed on PE as a DoubleRow matmul with a
    0/1 matrix (y-pair in the 2 planes, x-pair folded into the matrix),
    replacing the DVE add chain.
  - fc: chunk-pair DoubleRow matmuls (K=96 virtual) in an end-of-half burst so
    the logits PSUM bank borrows the conv2 pool's rotation.
  - every sign() is one PSUM->SBUF clamp/Sign instruction on [*,1024] tiles,
    alternating ACT and DVE to split the elementwise wall across both engines.
PSUM: conv1 pool 2x[128,1024] (4 banks) + conv2/pool/fc/transpose shared pool
2x[128,1024] (4 banks).
"""

import functools
import itertools as _it
import numpy as np
import ml_dtypes


def _chain(*gens):
    return _it.chain(*gens)

N_CORES = 8
B_TOTAL = 8192
B = B_TOTAL // N_CORES  # 1024 per core
BH = 512                # half-batch processed per outer iteration
THRESH = 0.2

FP8 = ml_dtypes.float8_e4m3


# ----------------------------------------------------------------------------
# Device program (built once, cached)
# ----------------------------------------------------------------------------

@functools.lru_cache(maxsize=1)
def _build_program():
    from contextlib import ExitStack
    import concourse.bass as bass
    import concourse.tile as tile
    import concourse.mybir as mybir
    from concourse import bacc

    f32 = mybir.dt.float32
    fp8 = mybir.dt.float8e4
    AF = mybir.ActivationFunctionType
    ALU = mybir.AluOpType
    AX = mybir.AxisListType
    DR = mybir.MatmulPerfMode.DoubleRow

    nc = bacc.Bacc(
        "TRN2",
        target_bir_lowering=False,
        debug=False,
        num_devices=N_CORES,
    )

    Y1 = 26          # conv1 out rows
    NW = 4           # x-windows
    WCOLS = Y1 * BH  # per-(w,h) window free size (13312)

    # host-prepacked, 2-replica conv1 window blocks: [8, 64, 26*512]
    xqr_t = nc.dram_tensor("xqr", [8, 64, WCOLS], fp8, kind="ExternalInput")
    wl1_t = nc.dram_tensor("wl1", [64, 128], fp8, kind="ExternalInput")
    wl2_t = nc.dram_tensor("wl2", [128, 384], fp8, kind="ExternalInput")
    wpl_t = nc.dram_tensor("wpl", [128, 96], fp8, kind="ExternalInput")
    wfc_t = nc.dram_tensor("wfc", [48, 768], fp8, kind="ExternalInput")
    idt_t = nc.dram_tensor("ident", [10, 10], f32, kind="ExternalInput")
    out_t = nc.dram_tensor("out", [B, 10], f32, kind="ExternalOutput")

    def emit(ctx, tc):
        wpool = ctx.enter_context(tc.tile_pool(name="weights", bufs=1))
        rhs1_pool = ctx.enter_context(tc.tile_pool(name="rhs1", bufs=2))
        a1_pool = ctx.enter_context(tc.tile_pool(name="a1", bufs=2))
        s2_pool = ctx.enter_context(tc.tile_pool(name="s2", bufs=2))
        psw_pool = ctx.enter_context(tc.tile_pool(name="psw", bufs=2))
        sm_pool = ctx.enter_context(tc.tile_pool(name="sm", bufs=10))
        ps_pool = ctx.enter_context(tc.tile_pool(name="ps", bufs=1, space="PSUM"))

        wl1 = wpool.tile([64, 128], fp8)
        nc.gpsimd.dma_start(wl1[:], wl1_t.ap())
        wl2 = wpool.tile([128, 384], fp8)
        nc.gpsimd.dma_start(wl2[:], wl2_t.ap())
        wpl = wpool.tile([128, 96], fp8)
        nc.gpsimd.dma_start(wpl[:], wpl_t.ap())
        wfc = wpool.tile([48, 768], fp8)
        nc.gpsimd.dma_start(wfc[:], wfc_t.ap())
        idt = wpool.tile([10, 10], f32)
        nc.gpsimd.dma_start(idt[:], idt_t.ap())
        lsb = wpool.tile([10, B], f32)  # logits staging, both halves

        eng = [0]

        def sign_to(dst, src):
            # src holds exact integers -> clamp(-1,1) == sign(); alternate
            # engines to split the PSUM->SBUF wall
            eng[0] ^= 1
            if eng[0]:
                nc.scalar.sign(dst, src)
            else:
                nc.vector.tensor_scalar(dst, src, -1.0, 1.0, ALU.max, ALU.min)

        def dma_rhs1(h, w):
            blk = h * NW + w
            rhs1 = rhs1_pool.tile([64, WCOLS], fp8, tag="rhs1", name="rhs1")
            for g in range(7):
                c0 = g * 2048
                cn = min(2048, WCOLS - c0)
                src = bass.AP(
                    xqr_t,
                    blk * 64 * WCOLS + c0,
                    [[WCOLS, 64], [1, cn]],
                )
                nc.sync.dma_start(rhs1[0:64, c0:c0 + cn], src)
            return rhs1

        def big_tile():
            return ps_pool.tile([128, 1024], f32, tag="big", name="bigt", bufs=3)

        def conv1_gen(rhs1, a1):
            # 13 row-tiled packs of 2 (y, y+1)
            for p in range(13):
                ps1 = big_tile()
                for i in range(2):
                    y = 2 * p + i
                    nc.tensor.matmul(
                        ps1[:, i * 512:(i + 1) * 512],
                        wl1[32 * i:32 * i + 30, :],
                        rhs1[32 * i:32 * i + 30, y * 512:(y + 1) * 512],
                        start=True, stop=True, tile_position=(32 * i, 0),
                    )
                sign_to(a1[:, p * 1024:(p + 1) * 1024], ps1[:])
                yield

        def conv2pool_gen(w, a1, s2, pswh):
            for q in range(6):        # py pairs
                pool_srcs = []
                for py in (2 * q, 2 * q + 1):
                    ps2 = big_tile()
                    for hy in range(2):
                        y = 2 * py + hy
                        nc.tensor.matmul(
                            ps2[:, hy * 512:(hy + 1) * 512],
                            wl2[:, 0:256].rearrange("p (two m) -> p two m", two=2),
                            a1[:, y * 512:(y + 2) * 512].rearrange(
                                "p (two n) -> p two n", two=2),
                            start=True, stop=False, perf_mode=DR,
                        )
                        nc.tensor.matmul(
                            ps2[:, hy * 512:(hy + 1) * 512],
                            wl2[:, 256:384],
                            a1[:, (y + 2) * 512:(y + 3) * 512],
                            start=False, stop=True,
                        )
                    sc = s2[:, py * 1024:(py + 1) * 1024]
                    sign_to(sc, ps2[:])
                    pool_srcs.append(sc)
                # pool: DR matmul per py (y-pair = 2 planes, x-pair in matrix)
                psp = big_tile()
                for j, sc in enumerate(pool_srcs):
                    nc.tensor.matmul(
                        psp[0:48, j * 512:(j + 1) * 512],
                        wpl[:].rearrange("p (two m) -> p two m", two=2),
                        sc.rearrange("p (two n) -> p two n", two=2),
                        start=True, stop=True, perf_mode=DR,
                    )
                sign_to(pswh[0:48, (w * 6 + q) * 1024:(w * 6 + q + 1) * 1024],
                        psp[0:48, :])
                yield

        def fc_gen(lg, pswh, j0, j1):
            for j in range(j0, j1):
                nc.tensor.matmul(
                    lg,
                    wfc[:, j * 32:(j + 1) * 32].rearrange(
                        "p (two m) -> p two m", two=2),
                    pswh[0:48, j * 1024:(j + 1) * 1024].rearrange(
                        "p (two n) -> p two n", two=2),
                    start=(j == 0), stop=(j == 23), perf_mode=DR,
                )
                yield

        def softmax_gen(h, lg):
            nc.vector.tensor_copy(lsb[:, h * BH:(h + 1) * BH], lg[0:10, :])
            yield
            # log_softmax on 4 chunks of 128 images, ACT funcs grouped
            lqs, nms, ses = [], [], []
            for qq in range(4):
                q = 4 * h + qq
                ptt = ps_pool.tile([128, 16], f32, tag="pt", name="ptt", bufs=1)
                nc.tensor.transpose(ptt[0:128, 0:10],
                                    lsb[:, q * 128:(q + 1) * 128], idt[:])
                lq = sm_pool.tile([128, 10], f32, tag=f"lq{qq}", name="lq")
                nc.vector.tensor_copy(lq[:], ptt[0:128, 0:10])
                nm = sm_pool.tile([128, 1], f32, tag=f"nm{qq}", name="nm")
                nc.vector.reduce_max(nm[:], lq[:], axis=AX.X, negate=True)
                lqs.append(lq)
                nms.append(nm)
                yield
            for qq in range(4):
                scr = sm_pool.tile([128, 10], f32, tag="scr", name="scr", bufs=2)
                se = sm_pool.tile([128, 1], f32, tag=f"se{qq}", name="se")
                nc.scalar.activation(scr[:], lqs[qq][:], AF.Exp,
                                     bias=nms[qq][:], accum_out=se[:])
                ses.append(se)
            yield
            lss = []
            for qq in range(4):
                ls = sm_pool.tile([128, 1], f32, tag=f"ls{qq}", name="ls")
                nc.scalar.activation(ls[:], ses[qq][:], AF.Ln)
                lss.append(ls)
            out_ap = out_t.ap()
            for qq in range(4):
                q = 4 * h + qq
                o = sm_pool.tile([128, 10], f32, tag="o", name="o", bufs=2)
                nc.vector.tensor_scalar(o[:], lqs[qq][:], nms[qq][:],
                                        lss[qq][:], ALU.add, ALU.subtract)
                nc.sync.dma_start(out_ap[q * 128:(q + 1) * 128, :], o[:])
                yield

        def drive(*pairs):
            active = [[g, wt] for g, wt in pairs if g is not None]
            while active:
                nxt = []
                for g, wt in active:
                    alive = True
                    for _ in range(wt):
                        try:
                            next(g)
                        except StopIteration:
                            alive = False
                            break
                    if alive:
                        nxt.append([g, wt])
                active = nxt

        tail = None
        for h in range(2):
            pswh = psw_pool.tile([48, 24 * 1024], fp8, tag="pswh", name="pswh")
            lgt = ps_pool.tile([16, 512], f32, tag="lg", name="lgt", bufs=1)
            lg = lgt[0:16, 0:512]
            rhs1 = dma_rhs1(h, 0)
            prev = None  # (w, a1, s2)
            for w in range(NW):
                a1 = a1_pool.tile([128, WCOLS], fp8, tag="a1", name="a1")
                c1 = conv1_gen(rhs1, a1)
                if w + 1 < NW:
                    rhs1 = dma_rhs1(h, w + 1)
                if prev is not None:
                    other = conv2pool_gen(prev[0], prev[1], prev[2], pswh)
                else:
                    other = tail  # fc tail + softmax of previous half
                drive((c1, 2), (other, 1))
                s2 = s2_pool.tile([128, 12 * 1024], fp8, tag="s2", name="s2")
                prev = (w, a1, s2)
            # last window's conv2/pool interleaved with fc of ready chunks
            drive((conv2pool_gen(prev[0], prev[1], prev[2], pswh), 1),
                  (fc_gen(lg, pswh, 0, 18), 3))
            tail = _chain(fc_gen(lg, pswh, 18, 24), softmax_gen(h, lg))
        drive((tail, 1))

    with tile.TileContext(nc) as tc:
        with ExitStack() as ctx:
            emit(ctx, tc)

    nc.compile()
    return nc


# ----------------------------------------------------------------------------
# Host-side packing
# ----------------------------------------------------------------------------

def _pack_weights(w1, w2, fc_w):
    w1s = np.sign(w1[:, 0].astype(np.float32))   # [16,3,3]
    w2s = np.sign(w2.astype(np.float32))         # [16,16,3,3]
    fcs = np.sign(fc_w.astype(np.float32))       # [10,2304]

    # conv1 Toeplitz: rows k=(dy,xi in 0..9), cols m=(o,xr in 0..7);
    # two replicas at partition bases 0 and 32 for row-tiling
    L1 = np.zeros((64, 128), np.float32)
    for o in range(16):
        for xr in range(8):
            for dy in range(3):
                for dx in range(3):
                    v = w1s[o, dy, dx]
                    L1[dy * 10 + xr + dx, o * 8 + xr] = v
                    L1[32 + dy * 10 + xr + dx, o * 8 + xr] = v

    # conv2 Toeplitz per dy: rows k=(c,xi in 0..7), cols j:
    #   j in [0,48):   o=j//3, xr=2*(j%3)      (even out-x)
    #   j in [64,112): o=(j-64)//3, xr=2*((j-64)%3)+1  (odd out-x)
    L2 = np.zeros((128, 384), np.float32)
    for dy in range(3):
        for c in range(16):
            for xi in range(8):
                k = c * 8 + xi
                for j in range(112):
                    if j < 48:
                        o, xr = j // 3, 2 * (j % 3)
                    elif j >= 64:
                        o, xr = (j - 64) // 3, 2 * ((j - 64) % 3) + 1
                    else:
                        continue
                    dx = xi - xr
                    if 0 <= dx <= 2:
                        if dy < 2:
                            L2[k, dy * 128 + j] = w2s[o, c, dy, dx]
                        else:
                            L2[k, 256 + j] = w2s[o, c, dy, dx]

    # pool matrix: out m=(o,pxl in 0..2) sums s2 partitions (even j, odd j);
    # DR: plane 0 and plane 1 identical (y-pair via rhs planes)
    P = np.zeros((128, 96), np.float32)
    for o in range(16):
        for pxl in range(3):
            m = o * 3 + pxl
            je = o * 3 + pxl          # even-x partition (j in [0,48))
            jo = 64 + o * 3 + pxl     # odd-x partition  (j in [64,112))
            for pl in range(2):
                P[je, pl * 48 + m] = 1.0
                P[jo, pl * 48 + m] = 1.0

    # fc chunk-pairs: pair j=(w*6+q) = chunks k0=(w,2q), k1=(w,2q+1),
    # k=(w,py): feature(p=(o,pxl)) = o*144 + py*12 + 3*w + pxl
    Lfc = np.zeros((48, 768), np.float32)
    for w in range(4):
        for q in range(6):
            j = w * 6 + q
            for pl in range(2):
                py = 2 * q + pl
                for p in range(48):
                    o, pxl = p // 3, p % 3
                    feat = o * 144 + py * 12 + 3 * w + pxl
                    Lfc[p, j * 32 + pl * 16:j * 32 + pl * 16 + 10] = fcs[:, feat]

    return (L1.astype(FP8), L2.astype(FP8), P.astype(FP8), Lfc.astype(FP8))


def _prep_inputs(x, w1, w2, fc_w):
    Y1 = 26
    xq = np.where(x.reshape(B_TOTAL, 28, 28) >= THRESH, 1.0, -1.0)
    xq_t = np.transpose(xq, (1, 2, 0)).astype(FP8)  # [28, 28, B_TOTAL]
    L1, L2, P, Lfc = _pack_weights(w1, w2, fc_w)
    ident = np.eye(10, dtype=np.float32)

    in_maps = []
    for i in range(N_CORES):
        xc = xq_t[:, :, i * B:(i + 1) * B]  # [28, 28, 1024]
        # window blocks: blk=(h,w): [64, 26*512] with taps (dy,xi) replicated
        # at partition bases 0 and 32; col (y,b) holds xq[y+dy, 6w+xi, h*512+b]
        xqr = np.zeros((8, 64, Y1 * BH), FP8)
        for h in range(2):
            for w in range(4):
                blk = h * 4 + w
                # [3dy, 10xi, 26y, 512b]
                base = np.stack([
                    np.stack([
                        xc[dy:dy + Y1, 6 * w + xi, h * BH:(h + 1) * BH]
                        for xi in range(10)
                    ], axis=0)
                    for dy in range(3)
                ], axis=0)
                flat = base.reshape(30, Y1 * BH)
                xqr[blk, 0:30] = flat
                xqr[blk, 32:62] = flat
        in_maps.append({
            "xqr": xqr, "wl1": L1, "wl2": L2, "wpl": P, "wfc": Lfc,
            "ident": ident,
        })
    return in_maps


# ----------------------------------------------------------------------------
# Entry point
# ----------------------------------------------------------------------------

TRACE = False
LAST_RESULTS = None


def kernel(x, w1, w2, fc_w):
    global LAST_RESULTS
    from concourse.bass_utils import run_bass_kernel_spmd

    x = np.asarray(x)
    in_maps = _prep_inputs(x, np.asarray(w1), np.asarray(w2), np.asarray(fc_w))
    nc = _build_program()
    res = run_bass_kernel_spmd(nc, in_maps, list(range(N_CORES)), trace=TRACE)
    LAST_RESULTS = res
    out = np.concatenate(
        [np.asarray(res.results[i]["out"]) for i in range(N_CORES)], axis=0
    )
    return out.astype(np.float32)


# revision 6
# speedup vs baseline: 1.7772x; 1.0938x over previous
"""Trainium2 Bass kernel for BinaryNN forward (binary conv net + log_softmax).

Contract: kernel(**inputs) takes FULL unsharded inputs
  x     [8192, 1, 28, 28] f32
  w1    [16, 1, 3, 3]     f32
  w2    [16, 16, 3, 3]    f32
  fc_w  [10, 2304]        f32
returns [8192, 10] f32 log_softmax logits.

Strategy: pure data parallel over 8 NeuronCores (batch 1024/core), conv lowered
to fp8 TensorEngine matmuls. v2 design from HW microbenchmarks:
  - conv1 (K=30) row-tiled 2x: two concurrent matmuls in PE row-groups 0/32
    (window data replicated at partition bases 0 and 32, pre-laid-out on host
    so the device DMA is a contiguous burst).
  - conv2: fp8 DoubleRow (dy0,dy1 as 2 K-planes in one pass, 2 planes/cycle)
    + single pass for dy2, per output row, N=512.
  - 2x2 avg-pool+sign: pool sums computed on PE as a DoubleRow matmul with a
    0/1 matrix (y-pair in the 2 planes, x-pair folded into the matrix),
    replacing the DVE add chain.
  - fc: chunk-pair DoubleRow matmuls (K=96 virtual) in an end-of-half burst so
    the logits PSUM bank borrows the conv2 pool's rotation.
  - every sign() is one PSUM->SBUF clamp/Sign instruction on [*,1024] tiles,
    alternating ACT and DVE to split the elementwise wall across both engines.
PSUM: conv1 pool 2x[128,1024] (4 banks) + conv2/pool/fc/transpose shared pool
2x[128,1024] (4 banks).
"""

import functools
import itertools as _it
import numpy as np
import ml_dtypes


def _chain(*gens):
    return _it.chain(*gens)

N_CORES = 8
B_TOTAL = 8192
B = B_TOTAL // N_CORES  # 1024 per core
BH = 512                # half-batch processed per outer iteration
THRESH = 0.2

FP8 = ml_dtypes.float8_e4m3


# ----------------------------------------------------------------------------
# Device program (built once, cached)
# ----------------------------------------------------------------------------

@functools.lru_cache(maxsize=1)
def _build_program():
    from contextlib import ExitStack
    import concourse.bass as bass
    import concourse.tile as tile
    import concourse.mybir as mybir
    from concourse import bacc

    f32 = mybir.dt.float32
    fp8 = mybir.dt.float8e4
    AF = mybir.ActivationFunctionType
    ALU = mybir.AluOpType
    AX = mybir.AxisListType
    DR = mybir.MatmulPerfMode.DoubleRow

    nc = bacc.Bacc(
        "TRN2",
        target_bir_lowering=False,
        debug=False,
        num_devices=N_CORES,
    )

    Y1 = 26          # conv1 out rows
    NW = 4           # x-windows
    WCOLS = Y1 * BH  # per-(w,h) window free size (13312)

    # host-prepacked, 2-replica conv1 window blocks: [8, 64, 26*512]
    xqr_t = nc.dram_tensor("xqr", [8, 128, WCOLS], fp8, kind="ExternalInput")
    wl1_t = nc.dram_tensor("wl1", [128, 128], fp8, kind="ExternalInput")
    wl2_t = nc.dram_tensor("wl2", [128, 384], fp8, kind="ExternalInput")
    wpl_t = nc.dram_tensor("wpl", [128, 96], fp8, kind="ExternalInput")
    wfc_t = nc.dram_tensor("wfc", [48, 768], fp8, kind="ExternalInput")
    idt_t = nc.dram_tensor("ident", [10, 10], f32, kind="ExternalInput")
    out_t = nc.dram_tensor("out", [B, 10], f32, kind="ExternalOutput")

    def emit(ctx, tc):
        wpool = ctx.enter_context(tc.tile_pool(name="weights", bufs=1))
        rhs1_pool = ctx.enter_context(tc.tile_pool(name="rhs1", bufs=2))
        a1_pool = ctx.enter_context(tc.tile_pool(name="a1", bufs=2))
        s2_pool = ctx.enter_context(tc.tile_pool(name="s2", bufs=2))
        psw_pool = ctx.enter_context(tc.tile_pool(name="psw", bufs=2))
        sm_pool = ctx.enter_context(tc.tile_pool(name="sm", bufs=10))
        ps_pool = ctx.enter_context(tc.tile_pool(name="ps", bufs=1, space="PSUM"))

        wl1 = wpool.tile([128, 128], fp8)
        nc.gpsimd.dma_start(wl1[:], wl1_t.ap())
        wl2 = wpool.tile([128, 384], fp8)
        nc.gpsimd.dma_start(wl2[:], wl2_t.ap())
        wpl = wpool.tile([128, 96], fp8)
        nc.gpsimd.dma_start(wpl[:], wpl_t.ap())
        wfc = wpool.tile([48, 768], fp8)
        nc.gpsimd.dma_start(wfc[:], wfc_t.ap())
        idt = wpool.tile([10, 10], f32)
        nc.gpsimd.dma_start(idt[:], idt_t.ap())
        lsb = wpool.tile([10, B], f32)  # logits staging, both halves

        eng = [0]

        def sign_to(dst, src):
            # src holds exact integers -> clamp(-1,1) == sign(); alternate
            # engines to split the PSUM->SBUF wall
            eng[0] ^= 1
            if eng[0]:
                nc.scalar.sign(dst, src)
            else:
                nc.vector.tensor_scalar(dst, src, -1.0, 1.0, ALU.max, ALU.min)

        def dma_rhs1(h, w):
            blk = h * NW + w
            rhs1 = rhs1_pool.tile([128, WCOLS], fp8, tag="rhs1", name="rhs1")
            for g in range(7):
                c0 = g * 2048
                cn = min(2048, WCOLS - c0)
                src = bass.AP(
                    xqr_t,
                    blk * 128 * WCOLS + c0,
                    [[WCOLS, 128], [1, cn]],
                )
                nc.sync.dma_start(rhs1[0:128, c0:c0 + cn], src)
            return rhs1

        def big_tile():
            return ps_pool.tile([128, 1024], f32, tag="big", name="bigt", bufs=3)

        def conv1_gen(rhs1, a1):
            # 7 row-tiled packs of 4 (last: 2), 4 concurrent row-groups
            for g in range(7):
                ny = min(4, Y1 - 4 * g)
                tiles = [big_tile() for _ in range((ny + 1) // 2)]
                for i in range(ny):
                    y = 4 * g + i
                    nc.tensor.matmul(
                        tiles[i // 2][:, (i % 2) * 512:(i % 2 + 1) * 512],
                        wl1[32 * i:32 * i + 30, :],
                        rhs1[32 * i:32 * i + 30, y * 512:(y + 1) * 512],
                        start=True, stop=True, tile_position=(32 * i, 0),
                    )
                for j, tl in enumerate(tiles):
                    sign_to(a1[:, (4 * g + 2 * j) * 512:
                               (4 * g + 2 * j + 2) * 512], tl[:])
                yield

        def emit_pool(w, q, pool_srcs, pswh):
            # pool: DR matmul per py (y-pair = 2 planes, x-pair in matrix)
            psp = big_tile()
            for j, sc in enumerate(pool_srcs):
                nc.tensor.matmul(
                    psp[0:48, j * 512:(j + 1) * 512],
                    wpl[:].rearrange("p (two m) -> p two m", two=2),
                    sc.rearrange("p (two n) -> p two n", two=2),
                    start=True, stop=True, perf_mode=DR,
                )
            sign_to(pswh[0:48, (w * 6 + q) * 1024:(w * 6 + q + 1) * 1024],
                    psp[0:48, :])

        def conv2pool_gen(w, a1, s2, pswh):
            pend = None
            for q in range(6):        # py pairs
                pool_srcs = []
                for py in (2 * q, 2 * q + 1):
                    ps2 = big_tile()
                    for hy in range(2):
                        y = 2 * py + hy
                        nc.tensor.matmul(
                            ps2[:, hy * 512:(hy + 1) * 512],
                            wl2[:, 0:256].rearrange("p (two m) -> p two m", two=2),
                            a1[:, y * 512:(y + 2) * 512].rearrange(
                                "p (two n) -> p two n", two=2),
                            start=True, stop=False, perf_mode=DR,
                        )
                        nc.tensor.matmul(
                            ps2[:, hy * 512:(hy + 1) * 512],
                            wl2[:, 256:384],
                            a1[:, (y + 2) * 512:(y + 3) * 512],
                            start=False, stop=True,
                        )
                    sc = s2[:, py * 1024:(py + 1) * 1024]
                    sign_to(sc, ps2[:])
                    pool_srcs.append(sc)
                if pend is not None:
                    emit_pool(w, pend[0], pend[1], pswh)
                pend = (q, pool_srcs)
                yield
            emit_pool(w, pend[0], pend[1], pswh)

        def fc_gen(lg, pswh, j0, j1):
            for j in range(j0, j1):
                nc.tensor.matmul(
                    lg,
                    wfc[:, j * 32:(j + 1) * 32].rearrange(
                        "p (two m) -> p two m", two=2),
                    pswh[0:48, j * 1024:(j + 1) * 1024].rearrange(
                        "p (two n) -> p two n", two=2),
                    start=(j == 0), stop=(j == 23), perf_mode=DR,
                )
                yield

        def softmax_gen(h, lg):
            nc.vector.tensor_copy(lsb[:, h * BH:(h + 1) * BH], lg[0:10, :])
            yield
            # log_softmax on 4 chunks of 128 images, ACT funcs grouped
            lqs, nms, ses = [], [], []
            for qq in range(4):
                q = 4 * h + qq
                ptt = ps_pool.tile([128, 16], f32, tag="pt", name="ptt", bufs=1)
                nc.tensor.transpose(ptt[0:128, 0:10],
                                    lsb[:, q * 128:(q + 1) * 128], idt[:])
                lq = sm_pool.tile([128, 10], f32, tag=f"lq{qq}", name="lq")
                nc.vector.tensor_copy(lq[:], ptt[0:128, 0:10])
                nm = sm_pool.tile([128, 1], f32, tag=f"nm{qq}", name="nm")
                nc.vector.reduce_max(nm[:], lq[:], axis=AX.X, negate=True)
                lqs.append(lq)
                nms.append(nm)
                yield
            for qq in range(4):
                scr = sm_pool.tile([128, 10], f32, tag="scr", name="scr", bufs=2)
                se = sm_pool.tile([128, 1], f32, tag=f"se{qq}", name="se")
                nc.scalar.activation(scr[:], lqs[qq][:], AF.Exp,
                                     bias=nms[qq][:], accum_out=se[:])
                ses.append(se)
            yield
            lss = []
            for qq in range(4):
                ls = sm_pool.tile([128, 1], f32, tag=f"ls{qq}", name="ls")
                nc.scalar.activation(ls[:], ses[qq][:], AF.Ln)
                lss.append(ls)
            out_ap = out_t.ap()
            for qq in range(4):
                q = 4 * h + qq
                o = sm_pool.tile([128, 10], f32, tag="o", name="o", bufs=2)
                nc.vector.tensor_scalar(o[:], lqs[qq][:], nms[qq][:],
                                        lss[qq][:], ALU.add, ALU.subtract)
                nc.sync.dma_start(out_ap[q * 128:(q + 1) * 128, :], o[:])
                yield

        def drive(*pairs):
            active = [[g, wt] for g, wt in pairs if g is not None]
            while active:
                nxt = []
                for g, wt in active:
                    alive = True
                    for _ in range(wt):
                        try:
                            next(g)
                        except StopIteration:
                            alive = False
                            break
                    if alive:
                        nxt.append([g, wt])
                active = nxt

        tail = None
        for h in range(2):
            pswh = psw_pool.tile([48, 24 * 1024], fp8, tag="pswh", name="pswh")
            lgt = ps_pool.tile([16, 512], f32, tag="lg", name="lgt", bufs=1)
            lg = lgt[0:16, 0:512]
            rhs1 = dma_rhs1(h, 0)
            prev = None  # (w, a1, s2)
            for w in range(NW):
                a1 = a1_pool.tile([128, WCOLS], fp8, tag="a1", name="a1")
                c1 = conv1_gen(rhs1, a1)
                if w + 1 < NW:
                    rhs1 = dma_rhs1(h, w + 1)
                if prev is not None:
                    other = conv2pool_gen(prev[0], prev[1], prev[2], pswh)
                else:
                    other = tail  # fc tail + softmax of previous half
                drive((c1, 1), (other, 1))
                s2 = s2_pool.tile([128, 12 * 1024], fp8, tag="s2", name="s2")
                prev = (w, a1, s2)
            # last window's conv2/pool interleaved with fc of ready chunks
            drive((conv2pool_gen(prev[0], prev[1], prev[2], pswh), 1),
                  (fc_gen(lg, pswh, 0, 18), 3))
            tail = _chain(fc_gen(lg, pswh, 18, 24), softmax_gen(h, lg))
        drive((tail, 1))

    with tile.TileContext(nc) as tc:
        with ExitStack() as ctx:
            emit(ctx, tc)

    nc.compile()
    return nc


# ----------------------------------------------------------------------------
# Host-side packing
# ----------------------------------------------------------------------------

def _pack_weights(w1, w2, fc_w):
    w1s = np.sign(w1[:, 0].astype(np.float32))   # [16,3,3]
    w2s = np.sign(w2.astype(np.float32))         # [16,16,3,3]
    fcs = np.sign(fc_w.astype(np.float32))       # [10,2304]

    # conv1 Toeplitz: rows k=(dy,xi in 0..9), cols m=(o,xr in 0..7);
    # two replicas at partition bases 0 and 32 for row-tiling
    L1 = np.zeros((128, 128), np.float32)
    for o in range(16):
        for xr in range(8):
            for dy in range(3):
                for dx in range(3):
                    v = w1s[o, dy, dx]
                    for r in range(4):
                        L1[32 * r + dy * 10 + xr + dx, o * 8 + xr] = v

    # conv2 Toeplitz per dy: rows k=(c,xi in 0..7), cols j:
    #   j in [0,48):   o=j//3, xr=2*(j%3)      (even out-x)
    #   j in [64,112): o=(j-64)//3, xr=2*((j-64)%3)+1  (odd out-x)
    L2 = np.zeros((128, 384), np.float32)
    for dy in range(3):
        for c in range(16):
            for xi in range(8):
                k = c * 8 + xi
                for j in range(112):
                    if j < 48:
                        o, xr = j // 3, 2 * (j % 3)
                    elif j >= 64:
                        o, xr = (j - 64) // 3, 2 * ((j - 64) % 3) + 1
                    else:
                        continue
                    dx = xi - xr
                    if 0 <= dx <= 2:
                        if dy < 2:
                            L2[k, dy * 128 + j] = w2s[o, c, dy, dx]
                        else:
                            L2[k, 256 + j] = w2s[o, c, dy, dx]

    # pool matrix: out m=(o,pxl in 0..2) sums s2 partitions (even j, odd j);
    # DR: plane 0 and plane 1 identical (y-pair via rhs planes)
    P = np.zeros((128, 96), np.float32)
    for o in range(16):
        for pxl in range(3):
            m = o * 3 + pxl
            je = o * 3 + pxl          # even-x partition (j in [0,48))
            jo = 64 + o * 3 + pxl     # odd-x partition  (j in [64,112))
            for pl in range(2):
                P[je, pl * 48 + m] = 1.0
                P[jo, pl * 48 + m] = 1.0

    # fc chunk-pairs: pair j=(w*6+q) = chunks k0=(w,2q), k1=(w,2q+1),
    # k=(w,py): feature(p=(o,pxl)) = o*144 + py*12 + 3*w + pxl
    Lfc = np.zeros((48, 768), np.float32)
    for w in range(4):
        for q in range(6):
            j = w * 6 + q
            for pl in range(2):
                py = 2 * q + pl
                for p in range(48):
                    o, pxl = p // 3, p % 3
                    feat = o * 144 + py * 12 + 3 * w + pxl
                    Lfc[p, j * 32 + pl * 16:j * 32 + pl * 16 + 10] = fcs[:, feat]

    return (L1.astype(FP8), L2.astype(FP8), P.astype(FP8), Lfc.astype(FP8))


def _prep_inputs(x, w1, w2, fc_w):
    Y1 = 26
    xq = np.where(x.reshape(B_TOTAL, 28, 28) >= THRESH, 1.0, -1.0)
    xq_t = np.transpose(xq, (1, 2, 0)).astype(FP8)  # [28, 28, B_TOTAL]
    L1, L2, P, Lfc = _pack_weights(w1, w2, fc_w)
    ident = np.eye(10, dtype=np.float32)

    in_maps = []
    for i in range(N_CORES):
        xc = xq_t[:, :, i * B:(i + 1) * B]  # [28, 28, 1024]
        # window blocks: blk=(h,w): [64, 26*512] with taps (dy,xi) replicated
        # at partition bases 0 and 32; col (y,b) holds xq[y+dy, 6w+xi, h*512+b]
        xqr = np.zeros((8, 128, Y1 * BH), FP8)
        for h in range(2):
            for w in range(4):
                blk = h * 4 + w
                # [3dy, 10xi, 26y, 512b]
                base = np.stack([
                    np.stack([
                        xc[dy:dy + Y1, 6 * w + xi, h * BH:(h + 1) * BH]
                        for xi in range(10)
                    ], axis=0)
                    for dy in range(3)
                ], axis=0)
                flat = base.reshape(30, Y1 * BH)
                for r in range(4):
                    xqr[blk, 32 * r:32 * r + 30] = flat
        in_maps.append({
            "xqr": xqr, "wl1": L1, "wl2": L2, "wpl": P, "wfc": Lfc,
            "ident": ident,
        })
    return in_maps


# ----------------------------------------------------------------------------
# Entry point
# ----------------------------------------------------------------------------

TRACE = False
LAST_RESULTS = None


def kernel(x, w1, w2, fc_w):
    global LAST_RESULTS
    from concourse.bass_utils import run_bass_kernel_spmd

    x = np.asarray(x)
    in_maps = _prep_inputs(x, np.asarray(w1), np.asarray(w2), np.asarray(fc_w))
    nc = _build_program()
    res = run_bass_kernel_spmd(nc, in_maps, list(range(N_CORES)), trace=TRACE)
    LAST_RESULTS = res
    out = np.concatenate(
        [np.asarray(res.results[i]["out"]) for i in range(N_CORES)], axis=0
    )
    return out.astype(np.float32)


# revision 12
# speedup vs baseline: 1.7873x; 1.0057x over previous
"""Trainium2 Bass kernel for BinaryNN forward (binary conv net + log_softmax).

Contract: kernel(**inputs) takes FULL unsharded inputs
  x     [8192, 1, 28, 28] f32
  w1    [16, 1, 3, 3]     f32
  w2    [16, 16, 3, 3]    f32
  fc_w  [10, 2304]        f32
returns [8192, 10] f32 log_softmax logits.

Strategy: pure data parallel over 8 NeuronCores (batch 1024/core), conv lowered
to fp8 TensorEngine matmuls. v2 design from HW microbenchmarks:
  - conv1 (K=30) row-tiled 2x: two concurrent matmuls in PE row-groups 0/32
    (window data replicated at partition bases 0 and 32, pre-laid-out on host
    so the device DMA is a contiguous burst).
  - conv2: fp8 DoubleRow (dy0,dy1 as 2 K-planes in one pass, 2 planes/cycle)
    + single pass for dy2, per output row, N=512.
  - 2x2 avg-pool+sign: pool sums computed on PE as a DoubleRow matmul with a
    0/1 matrix (y-pair in the 2 planes, x-pair folded into the matrix),
    replacing the DVE add chain.
  - fc: chunk-pair DoubleRow matmuls (K=96 virtual) in an end-of-half burst so
    the logits PSUM bank borrows the conv2 pool's rotation.
  - every sign() is one PSUM->SBUF clamp/Sign instruction on [*,1024] tiles,
    alternating ACT and DVE to split the elementwise wall across both engines.
PSUM: conv1 pool 2x[128,1024] (4 banks) + conv2/pool/fc/transpose shared pool
2x[128,1024] (4 banks).
"""

import functools
import itertools as _it
import numpy as np
import ml_dtypes


def _chain(*gens):
    return _it.chain(*gens)

N_CORES = 8
B_TOTAL = 8192
B = B_TOTAL // N_CORES  # 1024 per core
BH = 512                # half-batch processed per outer iteration
THRESH = 0.2

FP8 = ml_dtypes.float8_e4m3


# ----------------------------------------------------------------------------
# Device program (built once, cached)
# ----------------------------------------------------------------------------

@functools.lru_cache(maxsize=1)
def _build_program():
    from contextlib import ExitStack
    import concourse.bass as bass
    import concourse.tile as tile
    import concourse.mybir as mybir
    from concourse import bacc

    f32 = mybir.dt.float32
    fp8 = mybir.dt.float8e4
    AF = mybir.ActivationFunctionType
    ALU = mybir.AluOpType
    AX = mybir.AxisListType
    DR = mybir.MatmulPerfMode.DoubleRow

    nc = bacc.Bacc(
        "TRN2",
        target_bir_lowering=False,
        debug=False,
        num_devices=N_CORES,
    )

    Y1 = 26          # conv1 out rows
    NW = 4           # x-windows
    WCOLS = Y1 * BH  # per-(w,h) window free size (13312)

    # host-prepacked, 2-replica conv1 window blocks: [8, 64, 26*512]
    xqr_t = nc.dram_tensor("xqr", [8, 128, WCOLS], fp8, kind="ExternalInput")
    wl1_t = nc.dram_tensor("wl1", [128, 128], fp8, kind="ExternalInput")
    wl2_t = nc.dram_tensor("wl2", [128, 384], fp8, kind="ExternalInput")
    wpl_t = nc.dram_tensor("wpl", [128, 96], fp8, kind="ExternalInput")
    wfc_t = nc.dram_tensor("wfc", [48, 768], fp8, kind="ExternalInput")
    idt_t = nc.dram_tensor("ident", [10, 10], f32, kind="ExternalInput")
    out_t = nc.dram_tensor("out", [B, 10], f32, kind="ExternalOutput")

    def emit(ctx, tc):
        wpool = ctx.enter_context(tc.tile_pool(name="weights", bufs=1))
        rhs1_pool = ctx.enter_context(tc.tile_pool(name="rhs1", bufs=2))
        a1_pool = ctx.enter_context(tc.tile_pool(name="a1", bufs=2))
        s2_pool = ctx.enter_context(tc.tile_pool(name="s2", bufs=2))
        psw_pool = ctx.enter_context(tc.tile_pool(name="psw", bufs=2))
        sm_pool = ctx.enter_context(tc.tile_pool(name="sm", bufs=10))
        ps_pool = ctx.enter_context(tc.tile_pool(name="ps", bufs=1, space="PSUM"))

        wl1 = wpool.tile([128, 128], fp8)
        nc.gpsimd.dma_start(wl1[:], wl1_t.ap())
        wl2 = wpool.tile([128, 384], fp8)
        nc.gpsimd.dma_start(wl2[:], wl2_t.ap())
        wpl = wpool.tile([128, 96], fp8)
        nc.gpsimd.dma_start(wpl[:], wpl_t.ap())
        wfc = wpool.tile([48, 768], fp8)
        nc.gpsimd.dma_start(wfc[:], wfc_t.ap())
        idt = wpool.tile([10, 10], f32)
        nc.gpsimd.dma_start(idt[:], idt_t.ap())
        lsb = wpool.tile([10, B], f32)  # logits staging, both halves

        eng = [0]

        def sign_to(dst, src):
            # src holds exact integers -> clamp(-1,1) == sign(); alternate
            # engines to split the PSUM->SBUF wall
            eng[0] ^= 1
            if eng[0]:
                nc.scalar.sign(dst, src)
            else:
                nc.vector.tensor_scalar(dst, src, -1.0, 1.0, ALU.max, ALU.min)

        def dma_rhs1(h, w):
            blk = h * NW + w
            rhs1 = rhs1_pool.tile([128, WCOLS], fp8, tag="rhs1", name="rhs1")
            for g in range(7):
                c0 = g * 2048
                cn = min(2048, WCOLS - c0)
                src = bass.AP(
                    xqr_t,
                    blk * 128 * WCOLS + c0,
                    [[WCOLS, 128], [1, cn]],
                )
                nc.sync.dma_start(rhs1[0:128, c0:c0 + cn], src)
            return rhs1

        def big_tile():
            return ps_pool.tile([128, 1024], f32, tag="big", name="bigt", bufs=3)

        def conv1_gen(rhs1, a1):
            # 7 row-tiled packs of 4 (last: 2), 4 concurrent row-groups
            for g in range(7):
                ny = min(4, Y1 - 4 * g)
                tiles = [big_tile() for _ in range((ny + 1) // 2)]
                for i in range(ny):
                    y = 4 * g + i
                    nc.tensor.matmul(
                        tiles[i // 2][:, (i % 2) * 512:(i % 2 + 1) * 512],
                        wl1[32 * i:32 * i + 30, :],
                        rhs1[32 * i:32 * i + 30, y * 512:(y + 1) * 512],
                        start=True, stop=True, tile_position=(32 * i, 0),
                    )
                for j, tl in enumerate(tiles):
                    sign_to(a1[:, (4 * g + 2 * j) * 512:
                               (4 * g + 2 * j + 2) * 512], tl[:])
                yield

        def emit_pool(w, q, pool_srcs, pswh):
            # pool: DR matmul per py (y-pair = 2 planes, x-pair in matrix)
            psp = big_tile()
            for j, sc in enumerate(pool_srcs):
                nc.tensor.matmul(
                    psp[0:48, j * 512:(j + 1) * 512],
                    wpl[:].rearrange("p (two m) -> p two m", two=2),
                    sc.rearrange("p (two n) -> p two n", two=2),
                    start=True, stop=True, perf_mode=DR,
                )
            sign_to(pswh[0:48, (w * 6 + q) * 1024:(w * 6 + q + 1) * 1024],
                    psp[0:48, :])

        def conv2pool_gen(w, a1, s2, pswh):
            pend = None
            for q in range(6):        # py pairs
                pool_srcs = []
                for py in (2 * q, 2 * q + 1):
                    ps2 = big_tile()
                    for hy in range(2):
                        y = 2 * py + hy
                        nc.tensor.matmul(
                            ps2[:, hy * 512:(hy + 1) * 512],
                            wl2[:, 0:256].rearrange("p (two m) -> p two m", two=2),
                            a1[:, y * 512:(y + 2) * 512].rearrange(
                                "p (two n) -> p two n", two=2),
                            start=True, stop=False, perf_mode=DR,
                        )
                        nc.tensor.matmul(
                            ps2[:, hy * 512:(hy + 1) * 512],
                            wl2[:, 256:384],
                            a1[:, (y + 2) * 512:(y + 3) * 512],
                            start=False, stop=True,
                        )
                    sc = s2[:, py * 1024:(py + 1) * 1024]
                    sign_to(sc, ps2[:])
                    pool_srcs.append(sc)
                if pend is not None:
                    emit_pool(w, pend[0], pend[1], pswh)
                pend = (q, pool_srcs)
                yield
            emit_pool(w, pend[0], pend[1], pswh)

        def fc_gen(lg, pswh, j0, j1):
            for j in range(j0, j1):
                nc.tensor.matmul(
                    lg,
                    wfc[:, j * 32:(j + 1) * 32].rearrange(
                        "p (two m) -> p two m", two=2),
                    pswh[0:48, j * 1024:(j + 1) * 1024].rearrange(
                        "p (two n) -> p two n", two=2),
                    start=(j == 0), stop=(j == 23), perf_mode=DR,
                )
                yield

        def softmax_gen(h, lg):
            nc.vector.tensor_copy(lsb[:, h * BH:(h + 1) * BH], lg[0:10, :])
            yield
            # log_softmax on 4 chunks of 128 images, ACT funcs grouped
            lqs, nms, ses = [], [], []
            for qq in range(4):
                q = 4 * h + qq
                ptt = ps_pool.tile([128, 16], f32, tag="pt", name="ptt", bufs=1)
                nc.tensor.transpose(ptt[0:128, 0:10],
                                    lsb[:, q * 128:(q + 1) * 128], idt[:])
                lq = sm_pool.tile([128, 10], f32, tag=f"lq{qq}", name="lq")
                nc.vector.tensor_copy(lq[:], ptt[0:128, 0:10])
                nm = sm_pool.tile([128, 1], f32, tag=f"nm{qq}", name="nm")
                nc.vector.reduce_max(nm[:], lq[:], axis=AX.X, negate=True)
                lqs.append(lq)
                nms.append(nm)
                yield
            for qq in range(4):
                scr = sm_pool.tile([128, 10], f32, tag="scr", name="scr", bufs=2)
                se = sm_pool.tile([128, 1], f32, tag=f"se{qq}", name="se")
                nc.scalar.activation(scr[:], lqs[qq][:], AF.Exp,
                                     bias=nms[qq][:], accum_out=se[:])
                ses.append(se)
            yield
            lss = []
            for qq in range(4):
                ls = sm_pool.tile([128, 1], f32, tag=f"ls{qq}", name="ls")
                nc.scalar.activation(ls[:], ses[qq][:], AF.Ln)
                lss.append(ls)
            out_ap = out_t.ap()
            for qq in range(4):
                q = 4 * h + qq
                o = sm_pool.tile([128, 10], f32, tag="o", name="o", bufs=2)
                nc.vector.tensor_scalar(o[:], lqs[qq][:], nms[qq][:],
                                        lss[qq][:], ALU.add, ALU.subtract)
                nc.sync.dma_start(out_ap[q * 128:(q + 1) * 128, :], o[:])
                yield

        def drive(*pairs):
            active = [[g, wt] for g, wt in pairs if g is not None]
            while active:
                nxt = []
                for g, wt in active:
                    alive = True
                    for _ in range(wt):
                        try:
                            next(g)
                        except StopIteration:
                            alive = False
                            break
                    if alive:
                        nxt.append([g, wt])
                active = nxt

        tail = None
        for h in range(2):
            pswh = psw_pool.tile([48, 24 * 1024], fp8, tag="pswh", name="pswh")
            lgt = ps_pool.tile([16, 512], f32, tag="lg", name="lgt", bufs=1)
            lg = lgt[0:16, 0:512]
            rhs1 = dma_rhs1(h, 0)
            prev = None  # (w, a1, s2)
            for w in range(NW):
                a1 = a1_pool.tile([128, WCOLS], fp8, tag="a1", name="a1")
                c1 = conv1_gen(rhs1, a1)
                if w + 1 < NW:
                    rhs1 = dma_rhs1(h, w + 1)
                if prev is not None:
                    other = conv2pool_gen(prev[0], prev[1], prev[2], pswh)
                else:
                    other = tail  # fc tail + softmax of previous half
                drive((c1, 1), (other, 1))
                s2 = s2_pool.tile([128, 12 * 1024], fp8, tag="s2", name="s2")
                prev = (w, a1, s2)
            # last window's conv2/pool interleaved with fc of ready chunks
            drive((conv2pool_gen(prev[0], prev[1], prev[2], pswh), 1),
                  (fc_gen(lg, pswh, 0, 18), 3))
            tail = _chain(fc_gen(lg, pswh, 18, 24), softmax_gen(h, lg))
        drive((tail, 1))

    with tile.TileContext(nc) as tc:
        with ExitStack() as ctx:
            emit(ctx, tc)

    nc.compile()
    return nc


# ----------------------------------------------------------------------------
# Host-side packing
# ----------------------------------------------------------------------------

def _pack_weights(w1, w2, fc_w):
    w1s = np.sign(w1[:, 0].astype(np.float32))   # [16,3,3]
    w2s = np.sign(w2.astype(np.float32))         # [16,16,3,3]
    fcs = np.sign(fc_w.astype(np.float32))       # [10,2304]

    # conv1 Toeplitz: rows k=(dy,xi in 0..9), cols m=(o,xr in 0..7);
    # two replicas at partition bases 0 and 32 for row-tiling
    L1 = np.zeros((128, 128), np.float32)
    for o in range(16):
        for xr in range(8):
            for dy in range(3):
                for dx in range(3):
                    v = w1s[o, dy, dx]
                    for r in range(4):
                        L1[32 * r + dy * 10 + xr + dx, o * 8 + xr] = v

    # conv2 Toeplitz per dy: rows k=(c,xi in 0..7), cols j:
    #   j in [0,48):   o=j//3, xr=2*(j%3)      (even out-x)
    #   j in [64,112): o=(j-64)//3, xr=2*((j-64)%3)+1  (odd out-x)
    L2 = np.zeros((128, 384), np.float32)
    for dy in range(3):
        for c in range(16):
            for xi in range(8):
                k = c * 8 + xi
                for j in range(112):
                    if j < 48:
                        o, xr = j // 3, 2 * (j % 3)
                    elif j >= 64:
                        o, xr = (j - 64) // 3, 2 * ((j - 64) % 3) + 1
                    else:
                        continue
                    dx = xi - xr
                    if 0 <= dx <= 2:
                        if dy < 2:
                            L2[k, dy * 128 + j] = w2s[o, c, dy, dx]
                        else:
                            L2[k, 256 + j] = w2s[o, c, dy, dx]

    # pool matrix: out m=(o,pxl in 0..2) sums s2 partitions (even j, odd j);
    # DR: plane 0 and plane 1 identical (y-pair via rhs planes)
    P = np.zeros((128, 96), np.float32)
    for o in range(16):
        for pxl in range(3):
            m = o * 3 + pxl
            je = o * 3 + pxl          # even-x partition (j in [0,48))
            jo = 64 + o * 3 + pxl     # odd-x partition  (j in [64,112))
            for pl in range(2):
                P[je, pl * 48 + m] = 1.0
                P[jo, pl * 48 + m] = 1.0

    # fc chunk-pairs: pair j=(w*6+q) = chunks k0=(w,2q), k1=(w,2q+1),
    # k=(w,py): feature(p=(o,pxl)) = o*144 + py*12 + 3*w + pxl
    Lfc = np.zeros((48, 768), np.float32)
    for w in range(4):
        for q in range(6):
            j = w * 6 + q
            for pl in range(2):
                py = 2 * q + pl
                for p in range(48):
                    o, pxl = p // 3, p % 3
                    feat = o * 144 + py * 12 + 3 * w + pxl
                    Lfc[p, j * 32 + pl * 16:j * 32 + pl * 16 + 10] = fcs[:, feat]

    return (L1.astype(FP8), L2.astype(FP8), P.astype(FP8), Lfc.astype(FP8))


def _prep_inputs(x, w1, w2, fc_w):
    Y1 = 26
    xq = np.where(x.reshape(B_TOTAL, 28, 28) >= THRESH, 1.0, -1.0)
    xq_t = np.transpose(xq, (1, 2, 0)).astype(FP8)  # [28, 28, B_TOTAL]
    L1, L2, P, Lfc = _pack_weights(w1, w2, fc_w)
    ident = np.eye(10, dtype=np.float32)

    in_maps = []
    for i in range(N_CORES):
        xc = xq_t[:, :, i * B:(i + 1) * B]  # [28, 28, 1024]
        # window blocks: blk=(h,w): [64, 26*512] with taps (dy,xi) replicated
        # at partition bases 0 and 32; col (y,b) holds xq[y+dy, 6w+xi, h*512+b]
        xqr = np.zeros((8, 128, Y1 * BH), FP8)
        for h in range(2):
            for w in range(4):
                blk = h * 4 + w
                # [3dy, 10xi, 26y, 512b]
                base = np.stack([
                    np.stack([
                        xc[dy:dy + Y1, 6 * w + xi, h * BH:(h + 1) * BH]
                        for xi in range(10)
                    ], axis=0)
                    for dy in range(3)
                ], axis=0)
                flat = base.reshape(30, Y1 * BH)
                for r in range(4):
                    xqr[blk, 32 * r:32 * r + 30] = flat
        in_maps.append({
            "xqr": xqr, "wl1": L1, "wl2": L2, "wpl": P, "wfc": Lfc,
            "ident": ident,
        })
    return in_maps


# ----------------------------------------------------------------------------
# Entry point
# ----------------------------------------------------------------------------

TRACE = False
LAST_RESULTS = None


def kernel(x, w1, w2, fc_w):
    global LAST_RESULTS
    from concourse.bass_utils import run_bass_kernel_spmd

    x = np.asarray(x)
    in_maps = _prep_inputs(x, np.asarray(w1), np.asarray(w2), np.asarray(fc_w))
    nc = _build_program()
    res = run_bass_kernel_spmd(nc, in_maps, list(range(N_CORES)), trace=TRACE)
    LAST_RESULTS = res
    out = np.concatenate(
        [np.asarray(res.results[i]["out"]) for i in range(N_CORES)], axis=0
    )
    return out.astype(np.float32)
